# revision 82
# baseline (speedup 1.0000x reference)
"""Trainium2 Bass kernel for nn_Attention2D (sparse_attention), v3.

Data-parallel over rays across 8 cores. Per core, 64 tiles of 1024
view-tokens (tile = 8 blocks x 128 tokens; block = 16 rays x 8 views,
token col j = r*8+v). Pair-blocks i stack two blocks (sub s=0,1) into the
128-partition dim.

Math (s cancels in kh - qh; A_k = Wk.T@attn_w1, A_q = Wq.T@attn_w1,
P_a = pos_w2@attn_w1, c_z = pos_b2@attn_w1 + attn_b1):
  h1 = relu(k@A_k - q@A_q + hpos@P_a + c_z), hpos = relu(pos@w1+b1).
  Mask rides as hpos ch 8 (ones ch 9); masked tokens get h1 clipped to 0
  via +CLIP*(m-1), logits get +M_SHIFT*(m-1). u = k@Wv.T + hpos@pos_w2
  (bias s+pos_b2 folds into output bias). x = (sum_v u*e)/(sum_v e);
  out = x@out_w + b_out'.

v3 (95.8us cost-model, from 110.7us baseline):
  - k/q in fp8e4m3; z-path weights scaled by SZ=64 (e4m3 subnormal floor),
    un-scaled inside W3; u-path scaled by SU=64 in WV/UP, un-scaled in OW.
  - wv matmul in fp8 DoubleRow mode (k8a [64, (2s,4i,128j)] layout, half
    PE cost; DoubleRow only works at tile_position (0,0), so zk/zq stay
    classic fp8 with k8b/q8 [2s*64ch, .] layouts).
  - q8 deduplicated to per-ray (1/16 the bytes); zq rhs broadcasts the 8
    views via a stride-0 AP dim.
  - engine split: Act = exp + h1-relu + hpos-relu + obias; DVE = u*e mult
    + t1 + t2 + recip; Pool = dn + x. outT in bf16.
  - scheduling: lg/exp/mult lag one tile behind their producer so neither
    the 4-deep PE wait queue nor Act head-of-line-blocks younger work;
    PSUM dependency tracking is bank-granular, so z8 double-buffers across
    two real banks, u rotates 3 banks, and the out accumulator shares a
    bank only with the (off-critical-path) stage-1 psum. Startup DMAs are
    ordered so tile 0's zk dependencies (fp8 consts, first k chunk) land
    first; the final group's output DMA is split so half overlaps the
    last pairs' compute.
"""

import numpy as np
import ml_dtypes

BF16 = ml_dtypes.bfloat16
FP16 = np.float16
E4M3 = ml_dtypes.float8_e4m3
DIM, HID, B, N, V = 64, 8, 1024, 64, 8
NCORES = 8
B_C = B // NCORES
R_C = B_C * N              # 8192 rays per core
T_C = R_C * V              # 65536 view-tokens per core
TILE = 1024
NT_FULL = T_C // TILE      # 64 tiles
KCH = 4                    # tiles per k8a/k8b DMA chunk
QCH = 8                    # tiles per q8/posm DMA chunk
M_SHIFT = 8.0
CLIP = 8.0
SZ = 64.0                  # z-path fp8 weight scale
SU = 64.0                  # u-path fp8 weight scale

# bf16 consts column layout [128, CW]
C_ST1, C_ZP, C_W3, C_UP, C_OW = 0, 128, 256, 768, 1280
CW = 1408
# fp8 consts column layout [128, CW8]
F_ZK, F_ZQ, F_WV = 0, 32, 64
CW8 = 320

_PROG_CACHE: dict = {}


def _f32(x):
    return np.ascontiguousarray(np.asarray(x), dtype=np.float32)


# ----------------------------------------------------------------------------
# host-side preparation
# ----------------------------------------------------------------------------

def make_consts(inputs):
    """bf16 consts [128, CW], fp8 consts [128, CW8], f32 biases [128, 2]."""
    eid = int(np.asarray(inputs["embed_id1"]))
    Wq = _f32(inputs["q_tbl"])[eid].reshape(DIM, DIM)
    Wk = _f32(inputs["k_tbl"])[eid].reshape(DIM, DIM)
    Wv = _f32(inputs["v_tbl"])[eid].reshape(DIM, DIM)
    pos_w1, pos_b1 = _f32(inputs["pos_w1"]), _f32(inputs["pos_b1"])
    pos_w2, pos_b2 = _f32(inputs["pos_w2"]), _f32(inputs["pos_b2"])
    attn_w1, attn_b1 = _f32(inputs["attn_w1"]), _f32(inputs["attn_b1"])
    attn_w2, attn_b2 = _f32(inputs["attn_w2"]), _f32(inputs["attn_b2"])
    out_w, out_b = _f32(inputs["out_w"]), _f32(inputs["out_b"])

    s = _f32(inputs["strength"]) @ _f32(inputs["str_w"]) + _f32(inputs["str_b"])
    A_k = Wk.T @ attn_w1                          # [64, 8]
    A_q = Wq.T @ attn_w1
    P_a = pos_w2 @ attn_w1                        # [8, 8]
    c_z = pos_b2 @ attn_w1 + attn_b1              # [8]
    b_out = (s + pos_b2) @ out_w + out_b          # [64]

    C = np.zeros((128, CW), np.float32)
    # stage-1 lhsT [48, 128]: 8 diag blocks of [6, 16] (unscaled)
    for b in range(8):
        r, c = 6 * b, 16 * b
        C[r:r + 4, C_ST1 + c:C_ST1 + c + 8] = pos_w1
        C[r + 5, C_ST1 + c:C_ST1 + c + 8] = pos_b1
        C[r + 4, C_ST1 + c + 8] = 1.0             # mask carry
        C[r + 5, C_ST1 + c + 9] = 1.0             # ones carry
    # zp lhsT [128, 128]: 8 diag blocks of [16, 16], scaled by SZ
    for b in range(8):
        r = 16 * b
        C[r:r + 8, C_ZP + r:C_ZP + r + 8] = P_a * SZ
        C[r + 8, C_ZP + r:C_ZP + r + 8] = CLIP * SZ
        C[r + 8, C_ZP + r + 8] = SZ
        C[r + 9, C_ZP + r:C_ZP + r + 8] = (c_z - CLIP) * SZ
        C[r + 9, C_ZP + r + 9] = SZ
    # W3 lhsT: 4 K-padded [128, 128] blocks (pair i rows at 32i), /SZ
    for i in range(4):
        for sub in range(2):
            r, c = 32 * i + 16 * sub, 128 * i + 64 * sub
            C[r:r + 8, C_W3 + c:C_W3 + c + 64] = attn_w2 / SZ
            C[r + 8, C_W3 + c:C_W3 + c + 64] = M_SHIFT / SZ
            C[r + 9, C_W3 + c:C_W3 + c + 64] = attn_b2 / SZ
    # UP lhsT: 4 blocks, pos_w2 * SU
    for i in range(4):
        for sub in range(2):
            r, c = 32 * i + 16 * sub, 128 * i + 64 * sub
            C[r:r + 8, C_UP + c:C_UP + c + 64] = pos_w2 * SU
    # out_w / SU [128, 128]: 2 diag blocks of [64, 64]
    for sub in range(2):
        r = 64 * sub
        C[r:r + 64, C_OW + r:C_OW + r + 64] = out_w / SU

    C8 = np.zeros((128, CW8), np.float32)
    # zk / zq lhsT [128, 32]: 2 diag blocks of [64, 16], scaled by SZ
    for sub in range(2):
        r, c = 64 * sub, 16 * sub
        C8[r:r + 64, F_ZK + c:F_ZK + c + 8] = A_k * SZ
        C8[r:r + 64, F_ZQ + c:F_ZQ + c + 8] = -A_q * SZ
    # wv DoubleRow lhsT [64, (2 s, 128 m)]: Wv.T * SU on the s==s' diagonal
    for sub in range(2):
        C8[0:64, F_WV + sub * 128 + 64 * sub:F_WV + sub * 128 + 64 * sub + 64] = \
            Wv.T * SU

    biasf = np.zeros((128, 2), np.float32)
    biasf[:, 0] = -M_SHIFT
    for sub in range(2):
        biasf[64 * sub:64 * sub + 64, 1] = b_out
    return C.astype(BF16), C8.astype(E4M3), biasf


def prep_core(q, k, pos, mask_f, core):
    """Per-core transposed contiguous arrays."""
    b0 = core * B_C
    kc = _f32(k[b0:b0 + B_C]).reshape(T_C, DIM)
    qc = _f32(q[b0:b0 + B_C]).reshape(R_C, DIM)
    pc = _f32(pos[b0:b0 + B_C]).reshape(T_C, 4)
    mc = mask_f[b0:b0 + B_C].reshape(T_C)

    k5 = kc.reshape(NT_FULL, 4, 2, 128, DIM)          # (t, i, s, j, ch)
    # k8a[ch, t*1024 + s*512 + i*128 + j] (DoubleRow wv layout)
    k8a = np.ascontiguousarray(
        k5.transpose(4, 0, 2, 1, 3).reshape(DIM, NT_FULL * 1024).astype(E4M3))
    # k8b[s*64+ch, t*512 + i*128 + j] (classic zk layout)
    k8b = np.ascontiguousarray(
        k5.transpose(2, 4, 0, 1, 3).reshape(128, NT_FULL * 512).astype(E4M3))
    # q8[s*64+ch, t*64 + i*16 + r] (per-ray, views broadcast on-chip)
    q5 = qc.reshape(NT_FULL, 4, 2, 16, DIM)           # (t, i, s, r, ch)
    q8 = np.ascontiguousarray(
        q5.transpose(2, 4, 0, 1, 3).reshape(128, NT_FULL * 64).astype(E4M3))
    # posm[b*6+e, t*128 + j]: e 0-3 pos, 4 mask, 5 ones
    pm = np.empty((8, 6, NT_FULL, 128), np.float32)
    pm[:, 0:4] = pc.reshape(NT_FULL, 8, 128, 4).transpose(1, 3, 0, 2)
    pm[:, 4] = mc.reshape(NT_FULL, 8, 128).transpose(1, 0, 2)
    pm[:, 5] = 1.0
    posm = np.ascontiguousarray(pm.reshape(48, NT_FULL * 128).astype(BF16))
    return {"k8a": k8a, "k8b": k8b, "q8": q8, "posm": posm}


def unprep_out(outT):
    """outT [128, NT*64] -> [R_C, 64] ray-major."""
    v = _f32(outT).reshape(2, DIM, NT_FULL, 4, 16).transpose(2, 3, 0, 4, 1)
    return np.ascontiguousarray(v.reshape(R_C, DIM))


# ----------------------------------------------------------------------------
# device program
# ----------------------------------------------------------------------------

def build_program(nt=NT_FULL):
    if nt in _PROG_CACHE:
        return _PROG_CACHE[nt]

    import concourse.bacc as bacc
    import concourse.tile as tile
    import concourse.mybir as mybir

    f32 = mybir.dt.float32
    bf16 = mybir.dt.bfloat16
    fp8 = mybir.dt.float8e4
    nc = bacc.Bacc("TRN2", target_bir_lowering=False, debug=False,
                   enable_asserts=False, num_devices=NCORES)
    k8a_d = nc.dram_tensor("k8a", [64, nt * 1024], fp8, kind="ExternalInput").ap()
    k8b_d = nc.dram_tensor("k8b", [128, nt * 512], fp8, kind="ExternalInput").ap()
    q8_d = nc.dram_tensor("q8", [128, nt * 64], fp8, kind="ExternalInput").ap()
    posm_d = nc.dram_tensor("posm", [48, nt * 128], bf16, kind="ExternalInput").ap()
    cons_d = nc.dram_tensor("consts", [128, CW], bf16, kind="ExternalInput").ap()
    cons8_d = nc.dram_tensor("consts8", [128, CW8], fp8, kind="ExternalInput").ap()
    bias_d = nc.dram_tensor("biasf", [128, 2], f32, kind="ExternalInput").ap()
    outT_d = nc.dram_tensor("outT", [128, nt * 64], bf16, kind="ExternalOutput").ap()

    with tile.TileContext(nc) as tc:
        _emit(tc, nc, mybir, k8a_d, k8b_d, q8_d, posm_d, cons_d, cons8_d,
              bias_d, outT_d, nt)
    nc.compile()
    _PROG_CACHE[nt] = nc
    return nc


def _emit(tc, nc, mybir, k8a_d, k8b_d, q8_d, posm_d, cons_d, cons8_d,
          bias_d, outT_d, nt):
    from contextlib import ExitStack

    f32 = mybir.dt.float32
    bf16 = mybir.dt.bfloat16
    fp16 = mybir.dt.float16
    fp8 = mybir.dt.float8e4
    Exp = mybir.ActivationFunctionType.Exp
    Relu = mybir.ActivationFunctionType.Relu
    Ident = mybir.ActivationFunctionType.Identity
    add = mybir.AluOpType.add
    mult = mybir.AluOpType.mult
    DR = mybir.MatmulPerfMode.DoubleRow
    kch = min(KCH, nt)
    qch = min(QCH, nt)

    with ExitStack() as ctx:
        ep = ctx.enter_context
        cpool = ep(tc.tile_pool(name="consts", bufs=1))
        kapool = ep(tc.tile_pool(name="ka", bufs=3))
        kbpool = ep(tc.tile_pool(name="kb", bufs=3))
        qpool = ep(tc.tile_pool(name="qq", bufs=2))
        pmpool = ep(tc.tile_pool(name="pm", bufs=2))
        hppool = ep(tc.tile_pool(name="hp4", bufs=2))
        h1pool = ep(tc.tile_pool(name="h1", bufs=3))
        etpool = ep(tc.tile_pool(name="et", bufs=3))
        t1pool = ep(tc.tile_pool(name="t1", bufs=2))
        t2pool = ep(tc.tile_pool(name="t2", bufs=2))
        dnpool = ep(tc.tile_pool(name="dn", bufs=2))
        rxpool = ep(tc.tile_pool(name="rx", bufs=4))
        obpool = ep(tc.tile_pool(name="ob", bufs=2))
        pp_z = ep(tc.tile_pool(name="ps_z", bufs=2, space="PSUM"))
        pp_lg = ep(tc.tile_pool(name="ps_lg", bufs=2, space="PSUM"))
        pp_u = ep(tc.tile_pool(name="ps_u", bufs=3, space="PSUM"))
        pp_o = ep(tc.tile_pool(name="ps_o", bufs=1, space="PSUM"))

        # only the small fp8 consts load up front; the big bf16 consts and
        # biases are queued inside front(0) after the first k chunk so tile
        # 0's zk dependencies land as early as possible.
        cons8 = cpool.tile([128, CW8], fp8, tag="consts8")
        nc.sync.dma_start(cons8[:], cons8_d[:, :])
        biasf = cpool.tile([128, 2], f32, tag="biasf")
        cons = cpool.tile([128, CW], bf16, tag="consts")
        b_exp = biasf[:, 0:1]
        b_out = biasf[:, 1:2]

        lT1 = cons[0:48, C_ST1:C_ST1 + 128]
        lZP = cons[:, C_ZP:C_ZP + 128]
        lOW = cons[:, C_OW:C_OW + 128]
        lZK = cons8[:, F_ZK:F_ZK + 32]
        lZQ = cons8[:, F_ZQ:F_ZQ + 32]
        lWV = cons8[0:64, F_WV:F_WV + 256].rearrange("p (s m) -> p s m", s=2)

        # out accum [0:128] shares a bank with the stage-1 psum [128:384]
        ot = pp_o.tile([128, 512], f32, tag="ot")

        state = {}
        ets, us, h1s, obs = {}, {}, {}, {}

        def load_group(t0):
            """Queue q8/posm DMAs + ob tile for the 8-tile group at t0."""
            nq = min(qch, nt - t0)
            pb = pmpool.tile([48, qch * 128], bf16, tag="pb", name=f"pb{t0}")
            nc.sync.dma_start(pb[:, 0:nq * 128],
                              posm_d[:, t0 * 128:(t0 + nq) * 128])
            qq = qpool.tile([128, qch * 64], fp8, tag="qq", name=f"qq{t0}")
            nc.sync.dma_start(qq[:, 0:nq * 64],
                              q8_d[:, t0 * 64:(t0 + nq) * 64])
            ob = obpool.tile([128, 512], bf16, tag="ob", name=f"ob{t0}")
            obs[t0 // qch] = (ob, nq)
            return (qq, pb, ob, nq)

        def front(t):
            g, p = t // 2, t % 2
            # ---- input DMA management ----
            if t == 0:
                kb = kbpool.tile([128, kch * 512], fp8, tag="kb", name="kb0")
                nc.sync.dma_start(kb[:], k8b_d[:, 0:kch * 512])
                pb0 = pmpool.tile([48, qch * 128], bf16, tag="pb", name="pb0")
                nc.sync.dma_start(pb0[:], posm_d[:, 0:qch * 128])
                # stage1/zp weight columns first so tile 0's zp-chain starts
                # early; the bulky W3/UP/OW columns follow.
                nc.sync.dma_start(cons[:, 0:256], cons_d[:, 0:256])
                qq0 = qpool.tile([128, qch * 64], fp8, tag="qq", name="qq0")
                nc.sync.dma_start(qq0[:], q8_d[:, 0:qch * 64])
                nc.sync.dma_start(cons[:, 256:CW], cons_d[:, 256:CW])
                ob0 = obpool.tile([128, 512], bf16, tag="ob", name="ob0")
                obs[0] = (ob0, qch)
                state["grp"] = (qq0, pb0, ob0, qch)
                state["grp_n"] = None
                nc.sync.dma_start(biasf[:], bias_d[:, :])
                ka = kapool.tile([64, kch * 1024], fp8, tag="ka", name="ka0")
                nc.sync.dma_start(ka[:], k8a_d[:, 0:kch * 1024])
                state["ka"], state["kb"] = ka, kb
                state["ka_n"] = state["kb_n"] = None
            if t % kch == 1 and t + kch - 1 < nt:
                c0 = (t // kch + 1) * kch
                nb = min(kch, nt - c0)
                ka_n = kapool.tile([64, kch * 1024], fp8, tag="ka", name=f"ka{t}")
                nc.sync.dma_start(ka_n[:, 0:nb * 1024],
                                  k8a_d[:, c0 * 1024:(c0 + nb) * 1024])
                kb_n = kbpool.tile([128, kch * 512], fp8, tag="kb", name=f"kb{t}")
                nc.sync.dma_start(kb_n[:, 0:nb * 512],
                                  k8b_d[:, c0 * 512:(c0 + nb) * 512])
                state["ka_n"], state["kb_n"] = ka_n, kb_n
            if t % kch == 0 and t > 0:
                state["ka"], state["kb"] = state["ka_n"], state["kb_n"]
            if t % qch == 1 and t + qch - 1 < nt:
                state["grp_n"] = load_group((t // qch + 1) * qch)
            if t % qch == 0 and t > 0:
                state["grp"] = state["grp_n"]
            ka, kb = state["ka"], state["kb"]
            qq = state["grp"][0]

            # ---- stage 1 (hpos), produced one 2-tile window ahead ----
            def stage1(w0):
                grp = state["grp"] if (w0 // qch) == (t // qch) else state["grp_n"]
                pbw = grp[1]
                n2 = min(2, nt - w0)
                nc.tensor.matmul(ot[:, 128:128 + n2 * 128], lT1,
                                 pbw[:, (w0 % qch) * 128:(w0 % qch + n2) * 128],
                                 start=True, stop=True,
                                 tile_position=(0, 0), skip_group_check=True)
                hpos2 = hppool.tile([128, 256], bf16, tag="hpos2",
                                    name=f"hpos2_{w0}")
                nc.scalar.activation(hpos2[:, 0:n2 * 128], ot[:, 128:128 + n2 * 128],
                                     Relu)
                return hpos2

            if t == 0:
                state["hpos2"] = stage1(0)
            if t % 2 == 0 and t > 0:
                state["hpos2"] = state["hpos2_n"]
            if t % 2 == 0 and t + 2 < nt:
                state["hpos2_n"] = stage1(t + 2)
            hpos = state["hpos2"][:, (t % 2) * 128:(t % 2) * 128 + 128]

            # ---- z8 = zk + zq + zp (own PSUM bank per buffer) ----
            z8 = pp_z.tile([128, 128], f32, tag="z8", name=f"z8_{t}")
            kbt = kb[:, (t % kch) * 512:(t % kch) * 512 + 512]
            for i in range(4):
                nc.tensor.matmul(
                    z8[32 * i:32 * i + 32, :], lZK,
                    kbt[:, 128 * i:128 * i + 128], start=True, stop=False,
                    tile_position=(0, 32 * i), skip_group_check=True)
            for i in range(4):
                qsl = qq[:, (t % qch) * 64 + 16 * i:(t % qch) * 64 + 16 * i + 16]
                qb = qsl.unsqueeze(2).broadcast_to([128, 16, 8])
                nc.tensor.matmul(
                    z8[32 * i:32 * i + 32, :], lZQ, qb, start=False, stop=False,
                    tile_position=(0, 32 * i), skip_group_check=True)
            nc.tensor.matmul(z8, lZP, hpos, start=False, stop=True,
                             tile_position=(0, 0), skip_group_check=True)

            h1 = h1pool.tile([128, 128], bf16, tag="h1")
            nc.scalar.activation(h1[:], z8, Relu)
            h1s[t] = h1

            # et tile for the pair; lg/exp/mult of tile t-1 are emitted here so
            # neither the PE queue (lg waiting on h1) nor Act (exp waiting on
            # lg) ever head-of-line-blocks younger independent work. Emitting
            # them before wv also hides wv's u-buffer WAR wait under the lg
            # matmuls.
            if p == 0:
                state["et"] = etpool.tile([128, 2048], fp16, tag="et",
                                          name=f"et{t//2}")
            ets[g] = state["et"]
            emit_lg_exp_mult(t - 1)

            # ---- u = k@Wv.T + hpos@pos_w2 (scaled by SU) ----
            u = pp_u.tile([128, 512], f32, tag="u", name=f"u{t}")
            us[t] = u
            kat = ka[:, (t % kch) * 1024:(t % kch) * 1024 + 1024]
            kav = kat.rearrange("p (s i j) -> p s i j", s=2, i=4)
            nc.tensor.matmul(u[:], lWV, kav,
                             start=True, stop=False, perf_mode=DR,
                             tile_position=(0, 0), skip_group_check=True)
            for i in range(4):
                nc.tensor.matmul(
                    u[:, 128 * i:128 * i + 128],
                    cons[:, C_UP + 128 * i:C_UP + 128 * i + 128],
                    hpos, start=False, stop=True,
                    tile_position=(0, 0), skip_group_check=True)

        def emit_lg_exp_mult(t):
            if t < 0:
                return
            h1 = h1s.pop(t)
            lg = pp_lg.tile([128, 512], f32, tag="lg")
            for i in range(4):
                nc.tensor.matmul(
                    lg[:, 128 * i:128 * i + 128],
                    cons[:, C_W3 + 128 * i:C_W3 + 128 * i + 128],
                    h1[:], start=True, stop=True,
                    tile_position=(0, 0), skip_group_check=True)
            et = ets[t // 2]
            p = t % 2
            nc.scalar.activation(et[:, p * 512:p * 512 + 512],
                                 lg[:], Exp, bias=b_exp)
            u = us.pop(t)
            with nc.allow_low_precision(reason="fp16 softmax tail"):
                nc.vector.tensor_tensor(et[:, 1024 + p * 512:1536 + p * 512],
                                        u[:], et[:, p * 512:p * 512 + 512],
                                        mult)

        def tail_a(g, et):
            """Softmax tail (elementwise, post-mult) for tile pair g -> x.
            The final pair runs dn/x on DVE (idle by then, and ~1us faster
            than Pool) to shorten the drain."""
            last = g == nt // 2 - 1
            eng = nc.vector if last else nc.gpsimd
            with nc.allow_low_precision(reason="fp16 softmax tail"):
                etv = et[:].rearrange("p (gp v) -> p gp v", v=8)
                t1 = t1pool.tile([128, 1024], fp16, tag="t1")
                t1v = t1[:].rearrange("p (gp v) -> p gp v", v=4)
                nc.vector.tensor_tensor(t1v, etv[:, :, 0:4], etv[:, :, 4:8], add)
                t2 = t2pool.tile([128, 512], fp16, tag="t2")
                t2v = t2[:].rearrange("p (gp v) -> p gp v", v=2)
                nc.vector.tensor_tensor(t2v, t1v[:, :, 0:2], t1v[:, :, 2:4], add)
                dn = dnpool.tile([128, 256], f32, tag="dn")
                dnv = dn[:].rearrange("p (gp v) -> p gp v", v=1)
                eng.tensor_tensor(dnv, t2v[:, :, 0:1], t2v[:, :, 1:2], add)
            rden = rxpool.tile([128, 128], f32, tag="rden")
            nc.vector.reciprocal_approx_fast(rden[:], dn[:, 0:128])
            x = rxpool.tile([128, 128], bf16, tag="x")
            eng.tensor_tensor(x[:], dn[:, 128:256], rden[:], mult)
            return x

        def tail_b(g, x, ob, obn):
            """Output matmul + bias + DMA for tile pair g."""
            nc.tensor.matmul(ot[:, 0:128], lOW, x[:], start=True, stop=True,
                             tile_position=(0, 0), skip_group_check=True)
            nc.scalar.activation(ob[:, (g % 4) * 128:(g % 4) * 128 + 128],
                                 ot[:, 0:128], Ident, bias=b_out)
            last = 2 * g + 1 == nt - 1
            lastgrp = (g // 4) == (nt - 1) // 8
            if lastgrp and g % 4 == 1 and obn >= 4:
                g0 = (g // 4) * 8
                nc.sync.dma_start(outT_d[:, g0 * 64:(g0 + 4) * 64],
                                  ob[:, 0:256])
            if lastgrp and g % 4 == 2 and obn >= 6:
                g0 = (g // 4) * 8
                nc.sync.dma_start(outT_d[:, (g0 + 4) * 64:(g0 + 6) * 64],
                                  ob[:, 256:384])
            if g % 4 == 3 or last:
                g0 = (g // 4) * 8
                if lastgrp and obn >= 6:
                    nc.sync.dma_start(
                        outT_d[:, (g0 + 6) * 64:(g0 + obn) * 64],
                        ob[:, 384:obn * 64])
                elif lastgrp and obn >= 4:
                    nc.sync.dma_start(
                        outT_d[:, (g0 + 4) * 64:(g0 + obn) * 64],
                        ob[:, 256:obn * 64])
                else:
                    nc.sync.dma_start(outT_d[:, g0 * 64:(g0 + obn) * 64],
                                      ob[:, 0:obn * 64])

        # ---- software-pipelined main loop ----
        # front(t) emits exp(t-1); tail_a(g) runs once exp(2g+1) is emitted
        # (at t=2g+2); tail_b (out matmul) trails 2 more pairs so the PE
        # queue never blocks on the elementwise tail.
        pend = []

        def run_tail_a(g):
            x = tail_a(g, ets.pop(g))
            ob, obn = obs[(2 * g) // qch]
            pend.append((g, x, ob, obn))
            if len(pend) >= 3:
                tail_b(*pend.pop(0))

        for t in range(nt):
            front(t)
            if t % 2 == 1 and t >= 3:
                run_tail_a((t - 3) // 2)
        emit_lg_exp_mult(nt - 1)
        run_tail_a(nt // 2 - 1)
        while pend:
            tail_b(*pend.pop(0))


# ----------------------------------------------------------------------------
# entry point
# ----------------------------------------------------------------------------

def kernel(q, k, pos, strength, q_tbl, k_tbl, v_tbl,
           pos_w1, pos_b1, pos_w2, pos_b2,
           attn_w1, attn_b1, attn_w2, attn_b2,
           out_w, out_b, str_w, str_b, mask, embed_id1) -> np.ndarray:
    from concourse.bass_utils import run_bass_kernel_spmd

    inputs = dict(q=q, k=k, pos=pos, strength=strength, q_tbl=q_tbl,
                  k_tbl=k_tbl, v_tbl=v_tbl, pos_w1=pos_w1, pos_b1=pos_b1,
                  pos_w2=pos_w2, pos_b2=pos_b2, attn_w1=attn_w1,
                  attn_b1=attn_b1, attn_w2=attn_w2, attn_b2=attn_b2,
                  out_w=out_w, out_b=out_b, str_w=str_w, str_b=str_b,
                  mask=mask, embed_id1=embed_id1)
    nc = build_program(NT_FULL)
    consts, consts8, biasf = make_consts(inputs)
    mask_f = np.asarray(mask).astype(np.float32)
    in_maps = []
    for c in range(NCORES):
        m = prep_core(inputs["q"], inputs["k"], inputs["pos"], mask_f, c)
        m["consts"] = consts
        m["consts8"] = consts8
        m["biasf"] = biasf
        in_maps.append(m)
    res = run_bass_kernel_spmd(nc, in_maps, core_ids=list(range(NCORES)))
    out = np.empty((B * N, DIM), np.float32)
    for c in range(NCORES):
        out[c * R_C:(c + 1) * R_C] = unprep_out(res.results[c]["outT"])
    return out.reshape(B, N, DIM)


# revision 83
# speedup vs baseline: 1.0006x; 1.0006x over previous
"""Trainium2 Bass kernel for nn_Attention2D (sparse_attention), v3.

Data-parallel over rays across 8 cores. Per core, 64 tiles of 1024
view-tokens (tile = 8 blocks x 128 tokens; block = 16 rays x 8 views,
token col j = r*8+v). Pair-blocks i stack two blocks (sub s=0,1) into the
128-partition dim.

Math (s cancels in kh - qh; A_k = Wk.T@attn_w1, A_q = Wq.T@attn_w1,
P_a = pos_w2@attn_w1, c_z = pos_b2@attn_w1 + attn_b1):
  h1 = relu(k@A_k - q@A_q + hpos@P_a + c_z), hpos = relu(pos@w1+b1).
  Mask rides as hpos ch 8 (ones ch 9); masked tokens get h1 clipped to 0
  via +CLIP*(m-1), logits get +M_SHIFT*(m-1). u = k@Wv.T + hpos@pos_w2
  (bias s+pos_b2 folds into output bias). x = (sum_v u*e)/(sum_v e);
  out = x@out_w + b_out'.

v3 (95.8us cost-model, from 110.7us baseline):
  - k/q in fp8e4m3; z-path weights scaled by SZ=64 (e4m3 subnormal floor),
    un-scaled inside W3; u-path scaled by SU=64 in WV/UP, un-scaled in OW.
  - wv matmul in fp8 DoubleRow mode (k8a [64, (2s,4i,128j)] layout, half
    PE cost; DoubleRow only works at tile_position (0,0), so zk/zq stay
    classic fp8 with k8b/q8 [2s*64ch, .] layouts).
  - q8 deduplicated to per-ray (1/16 the bytes); zq rhs broadcasts the 8
    views via a stride-0 AP dim.
  - engine split: Act = exp + h1-relu + hpos-relu + obias; DVE = u*e mult
    + t1 + t2 + recip; Pool = dn + x. outT in bf16.
  - scheduling: lg/exp/mult lag one tile behind their producer so neither
    the 4-deep PE wait queue nor Act head-of-line-blocks younger work;
    PSUM dependency tracking is bank-granular, so z8 double-buffers across
    two real banks, u rotates 3 banks, and the out accumulator shares a
    bank only with the (off-critical-path) stage-1 psum. Startup DMAs are
    ordered so tile 0's zk dependencies (fp8 consts, first k chunk) land
    first; the final group's output DMA is split so half overlaps the
    last pairs' compute.
"""

import numpy as np
import ml_dtypes

BF16 = ml_dtypes.bfloat16
FP16 = np.float16
E4M3 = ml_dtypes.float8_e4m3
DIM, HID, B, N, V = 64, 8, 1024, 64, 8
NCORES = 8
B_C = B // NCORES
R_C = B_C * N              # 8192 rays per core
T_C = R_C * V              # 65536 view-tokens per core
TILE = 1024
NT_FULL = T_C // TILE      # 64 tiles
KCH = 4                    # tiles per k8a/k8b DMA chunk
QCH = 8                    # tiles per q8/posm DMA chunk
M_SHIFT = 8.0
CLIP = 8.0
SZ = 64.0                  # z-path fp8 weight scale
SU = 64.0                  # u-path fp8 weight scale

# bf16 consts column layout [128, CW]
C_ST1, C_ZP, C_W3, C_UP, C_OW = 0, 128, 256, 768, 1280
CW = 1408
# fp8 consts column layout [128, CW8]
F_ZK, F_ZQ, F_WV = 0, 32, 64
CW8 = 320

_PROG_CACHE: dict = {}


def _f32(x):
    return np.ascontiguousarray(np.asarray(x), dtype=np.float32)


# ----------------------------------------------------------------------------
# host-side preparation
# ----------------------------------------------------------------------------

def make_consts(inputs):
    """bf16 consts [128, CW], fp8 consts [128, CW8], f32 biases [128, 2]."""
    eid = int(np.asarray(inputs["embed_id1"]))
    Wq = _f32(inputs["q_tbl"])[eid].reshape(DIM, DIM)
    Wk = _f32(inputs["k_tbl"])[eid].reshape(DIM, DIM)
    Wv = _f32(inputs["v_tbl"])[eid].reshape(DIM, DIM)
    pos_w1, pos_b1 = _f32(inputs["pos_w1"]), _f32(inputs["pos_b1"])
    pos_w2, pos_b2 = _f32(inputs["pos_w2"]), _f32(inputs["pos_b2"])
    attn_w1, attn_b1 = _f32(inputs["attn_w1"]), _f32(inputs["attn_b1"])
    attn_w2, attn_b2 = _f32(inputs["attn_w2"]), _f32(inputs["attn_b2"])
    out_w, out_b = _f32(inputs["out_w"]), _f32(inputs["out_b"])

    s = _f32(inputs["strength"]) @ _f32(inputs["str_w"]) + _f32(inputs["str_b"])
    A_k = Wk.T @ attn_w1                          # [64, 8]
    A_q = Wq.T @ attn_w1
    P_a = pos_w2 @ attn_w1                        # [8, 8]
    c_z = pos_b2 @ attn_w1 + attn_b1              # [8]
    b_out = (s + pos_b2) @ out_w + out_b          # [64]

    C = np.zeros((128, CW), np.float32)
    # stage-1 lhsT [48, 128]: 8 diag blocks of [6, 16] (unscaled)
    for b in range(8):
        r, c = 6 * b, 16 * b
        C[r:r + 4, C_ST1 + c:C_ST1 + c + 8] = pos_w1
        C[r + 5, C_ST1 + c:C_ST1 + c + 8] = pos_b1
        C[r + 4, C_ST1 + c + 8] = 1.0             # mask carry
        C[r + 5, C_ST1 + c + 9] = 1.0             # ones carry
    # zp lhsT [128, 128]: 8 diag blocks of [16, 16], scaled by SZ
    for b in range(8):
        r = 16 * b
        C[r:r + 8, C_ZP + r:C_ZP + r + 8] = P_a * SZ
        C[r + 8, C_ZP + r:C_ZP + r + 8] = CLIP * SZ
        C[r + 8, C_ZP + r + 8] = SZ
        C[r + 9, C_ZP + r:C_ZP + r + 8] = (c_z - CLIP) * SZ
        C[r + 9, C_ZP + r + 9] = SZ
    # W3 lhsT: 4 K-padded [128, 128] blocks (pair i rows at 32i), /SZ
    for i in range(4):
        for sub in range(2):
            r, c = 32 * i + 16 * sub, 128 * i + 64 * sub
            C[r:r + 8, C_W3 + c:C_W3 + c + 64] = attn_w2 / SZ
            C[r + 8, C_W3 + c:C_W3 + c + 64] = M_SHIFT / SZ
            C[r + 9, C_W3 + c:C_W3 + c + 64] = attn_b2 / SZ
    # UP lhsT: 4 blocks, pos_w2 * SU
    for i in range(4):
        for sub in range(2):
            r, c = 32 * i + 16 * sub, 128 * i + 64 * sub
            C[r:r + 8, C_UP + c:C_UP + c + 64] = pos_w2 * SU
    # out_w / SU [128, 128]: 2 diag blocks of [64, 64]
    for sub in range(2):
        r = 64 * sub
        C[r:r + 64, C_OW + r:C_OW + r + 64] = out_w / SU

    C8 = np.zeros((128, CW8), np.float32)
    # zk / zq lhsT [128, 32]: 2 diag blocks of [64, 16], scaled by SZ
    for sub in range(2):
        r, c = 64 * sub, 16 * sub
        C8[r:r + 64, F_ZK + c:F_ZK + c + 8] = A_k * SZ
        C8[r:r + 64, F_ZQ + c:F_ZQ + c + 8] = -A_q * SZ
    # wv DoubleRow lhsT [64, (2 s, 128 m)]: Wv.T * SU on the s==s' diagonal
    for sub in range(2):
        C8[0:64, F_WV + sub * 128 + 64 * sub:F_WV + sub * 128 + 64 * sub + 64] = \
            Wv.T * SU

    biasf = np.zeros((128, 2), np.float32)
    biasf[:, 0] = -M_SHIFT
    for sub in range(2):
        biasf[64 * sub:64 * sub + 64, 1] = b_out
    return C.astype(BF16), C8.astype(E4M3), biasf


def prep_core(q, k, pos, mask_f, core):
    """Per-core transposed contiguous arrays."""
    b0 = core * B_C
    kc = _f32(k[b0:b0 + B_C]).reshape(T_C, DIM)
    qc = _f32(q[b0:b0 + B_C]).reshape(R_C, DIM)
    pc = _f32(pos[b0:b0 + B_C]).reshape(T_C, 4)
    mc = mask_f[b0:b0 + B_C].reshape(T_C)

    k5 = kc.reshape(NT_FULL, 4, 2, 128, DIM)          # (t, i, s, j, ch)
    # k8a[ch, t*1024 + s*512 + i*128 + j] (DoubleRow wv layout)
    k8a = np.ascontiguousarray(
        k5.transpose(4, 0, 2, 1, 3).reshape(DIM, NT_FULL * 1024).astype(E4M3))
    # k8b[s*64+ch, t*512 + i*128 + j] (classic zk layout)
    k8b = np.ascontiguousarray(
        k5.transpose(2, 4, 0, 1, 3).reshape(128, NT_FULL * 512).astype(E4M3))
    # q8[s*64+ch, t*64 + i*16 + r] (per-ray, views broadcast on-chip)
    q5 = qc.reshape(NT_FULL, 4, 2, 16, DIM)           # (t, i, s, r, ch)
    q8 = np.ascontiguousarray(
        q5.transpose(2, 4, 0, 1, 3).reshape(128, NT_FULL * 64).astype(E4M3))
    # posm[b*6+e, t*128 + j]: e 0-3 pos, 4 mask, 5 ones
    pm = np.empty((8, 6, NT_FULL, 128), np.float32)
    pm[:, 0:4] = pc.reshape(NT_FULL, 8, 128, 4).transpose(1, 3, 0, 2)
    pm[:, 4] = mc.reshape(NT_FULL, 8, 128).transpose(1, 0, 2)
    pm[:, 5] = 1.0
    posm = np.ascontiguousarray(pm.reshape(48, NT_FULL * 128).astype(BF16))
    return {"k8a": k8a, "k8b": k8b, "q8": q8, "posm": posm}


def unprep_out(outT):
    """outT [128, NT*64] -> [R_C, 64] ray-major."""
    v = _f32(outT).reshape(2, DIM, NT_FULL, 4, 16).transpose(2, 3, 0, 4, 1)
    return np.ascontiguousarray(v.reshape(R_C, DIM))


# ----------------------------------------------------------------------------
# device program
# ----------------------------------------------------------------------------

def build_program(nt=NT_FULL):
    if nt in _PROG_CACHE:
        return _PROG_CACHE[nt]

    import concourse.bacc as bacc
    import concourse.tile as tile
    import concourse.mybir as mybir

    f32 = mybir.dt.float32
    bf16 = mybir.dt.bfloat16
    fp8 = mybir.dt.float8e4
    nc = bacc.Bacc("TRN2", target_bir_lowering=False, debug=False,
                   enable_asserts=False, num_devices=NCORES)
    k8a_d = nc.dram_tensor("k8a", [64, nt * 1024], fp8, kind="ExternalInput").ap()
    k8b_d = nc.dram_tensor("k8b", [128, nt * 512], fp8, kind="ExternalInput").ap()
    q8_d = nc.dram_tensor("q8", [128, nt * 64], fp8, kind="ExternalInput").ap()
    posm_d = nc.dram_tensor("posm", [48, nt * 128], bf16, kind="ExternalInput").ap()
    cons_d = nc.dram_tensor("consts", [128, CW], bf16, kind="ExternalInput").ap()
    cons8_d = nc.dram_tensor("consts8", [128, CW8], fp8, kind="ExternalInput").ap()
    bias_d = nc.dram_tensor("biasf", [128, 2], f32, kind="ExternalInput").ap()
    outT_d = nc.dram_tensor("outT", [128, nt * 64], bf16, kind="ExternalOutput").ap()

    with tile.TileContext(nc) as tc:
        _emit(tc, nc, mybir, k8a_d, k8b_d, q8_d, posm_d, cons_d, cons8_d,
              bias_d, outT_d, nt)
    nc.compile()
    _PROG_CACHE[nt] = nc
    return nc


def _emit(tc, nc, mybir, k8a_d, k8b_d, q8_d, posm_d, cons_d, cons8_d,
          bias_d, outT_d, nt):
    from contextlib import ExitStack

    f32 = mybir.dt.float32
    bf16 = mybir.dt.bfloat16
    fp16 = mybir.dt.float16
    fp8 = mybir.dt.float8e4
    Exp = mybir.ActivationFunctionType.Exp
    Relu = mybir.ActivationFunctionType.Relu
    Ident = mybir.ActivationFunctionType.Identity
    add = mybir.AluOpType.add
    mult = mybir.AluOpType.mult
    DR = mybir.MatmulPerfMode.DoubleRow
    kch = min(KCH, nt)
    qch = min(QCH, nt)

    with ExitStack() as ctx:
        ep = ctx.enter_context
        cpool = ep(tc.tile_pool(name="consts", bufs=1))
        kapool = ep(tc.tile_pool(name="ka", bufs=3))
        kbpool = ep(tc.tile_pool(name="kb", bufs=3))
        qpool = ep(tc.tile_pool(name="qq", bufs=2))
        pmpool = ep(tc.tile_pool(name="pm", bufs=2))
        hppool = ep(tc.tile_pool(name="hp4", bufs=2))
        h1pool = ep(tc.tile_pool(name="h1", bufs=3))
        etpool = ep(tc.tile_pool(name="et", bufs=3))
        t1pool = ep(tc.tile_pool(name="t1", bufs=2))
        t2pool = ep(tc.tile_pool(name="t2", bufs=2))
        dnpool = ep(tc.tile_pool(name="dn", bufs=2))
        rxpool = ep(tc.tile_pool(name="rx", bufs=4))
        obpool = ep(tc.tile_pool(name="ob", bufs=2))
        pp_z = ep(tc.tile_pool(name="ps_z", bufs=2, space="PSUM"))
        pp_lg = ep(tc.tile_pool(name="ps_lg", bufs=2, space="PSUM"))
        pp_u = ep(tc.tile_pool(name="ps_u", bufs=3, space="PSUM"))
        pp_o = ep(tc.tile_pool(name="ps_o", bufs=1, space="PSUM"))

        # only the small fp8 consts load up front; the big bf16 consts and
        # biases are queued inside front(0) after the first k chunk so tile
        # 0's zk dependencies land as early as possible.
        cons8 = cpool.tile([128, CW8], fp8, tag="consts8")
        nc.sync.dma_start(cons8[:], cons8_d[:, :])
        biasf = cpool.tile([128, 2], f32, tag="biasf")
        cons = cpool.tile([128, CW], bf16, tag="consts")
        b_exp = biasf[:, 0:1]
        b_out = biasf[:, 1:2]

        lT1 = cons[0:48, C_ST1:C_ST1 + 128]
        lZP = cons[:, C_ZP:C_ZP + 128]
        lOW = cons[:, C_OW:C_OW + 128]
        lZK = cons8[:, F_ZK:F_ZK + 32]
        lZQ = cons8[:, F_ZQ:F_ZQ + 32]
        lWV = cons8[0:64, F_WV:F_WV + 256].rearrange("p (s m) -> p s m", s=2)

        # out accum [0:128] shares a bank with the stage-1 psum [128:384]
        ot = pp_o.tile([128, 512], f32, tag="ot")

        state = {}
        ets, us, h1s, obs = {}, {}, {}, {}

        def load_group(t0):
            """Queue q8/posm DMAs + ob tile for the 8-tile group at t0."""
            nq = min(qch, nt - t0)
            pb = pmpool.tile([48, qch * 128], bf16, tag="pb", name=f"pb{t0}")
            nc.sync.dma_start(pb[:, 0:nq * 128],
                              posm_d[:, t0 * 128:(t0 + nq) * 128])
            qq = qpool.tile([128, qch * 64], fp8, tag="qq", name=f"qq{t0}")
            nc.sync.dma_start(qq[:, 0:nq * 64],
                              q8_d[:, t0 * 64:(t0 + nq) * 64])
            ob = obpool.tile([128, 512], bf16, tag="ob", name=f"ob{t0}")
            obs[t0 // qch] = (ob, nq)
            return (qq, pb, ob, nq)

        def front(t):
            g, p = t // 2, t % 2
            # ---- input DMA management ----
            if t == 0:
                kb = kbpool.tile([128, kch * 512], fp8, tag="kb", name="kb0")
                nc.sync.dma_start(kb[:], k8b_d[:, 0:kch * 512])
                pb0 = pmpool.tile([48, qch * 128], bf16, tag="pb", name="pb0")
                nc.sync.dma_start(pb0[:], posm_d[:, 0:qch * 128])
                # stage1/zp weight columns first so tile 0's zp-chain starts
                # early; the bulky W3/UP/OW columns follow.
                nc.sync.dma_start(cons[:, 0:256], cons_d[:, 0:256])
                qq0 = qpool.tile([128, qch * 64], fp8, tag="qq", name="qq0")
                nc.sync.dma_start(qq0[:], q8_d[:, 0:qch * 64])
                nc.sync.dma_start(cons[:, 256:CW], cons_d[:, 256:CW])
                ob0 = obpool.tile([128, 512], bf16, tag="ob", name="ob0")
                obs[0] = (ob0, qch)
                state["grp"] = (qq0, pb0, ob0, qch)
                state["grp_n"] = None
                ka = kapool.tile([64, kch * 1024], fp8, tag="ka", name="ka0")
                nc.sync.dma_start(ka[:], k8a_d[:, 0:kch * 1024])
                nc.sync.dma_start(biasf[:], bias_d[:, :])
                state["ka"], state["kb"] = ka, kb
                state["ka_n"] = state["kb_n"] = None
            if t % kch == 1 and t + kch - 1 < nt:
                c0 = (t // kch + 1) * kch
                nb = min(kch, nt - c0)
                ka_n = kapool.tile([64, kch * 1024], fp8, tag="ka", name=f"ka{t}")
                nc.sync.dma_start(ka_n[:, 0:nb * 1024],
                                  k8a_d[:, c0 * 1024:(c0 + nb) * 1024])
                kb_n = kbpool.tile([128, kch * 512], fp8, tag="kb", name=f"kb{t}")
                nc.sync.dma_start(kb_n[:, 0:nb * 512],
                                  k8b_d[:, c0 * 512:(c0 + nb) * 512])
                state["ka_n"], state["kb_n"] = ka_n, kb_n
            if t % kch == 0 and t > 0:
                state["ka"], state["kb"] = state["ka_n"], state["kb_n"]
            if t % qch == 1 and t + qch - 1 < nt:
                state["grp_n"] = load_group((t // qch + 1) * qch)
            if t % qch == 0 and t > 0:
                state["grp"] = state["grp_n"]
            ka, kb = state["ka"], state["kb"]
            qq = state["grp"][0]

            # ---- stage 1 (hpos), produced one 2-tile window ahead ----
            def stage1(w0):
                grp = state["grp"] if (w0 // qch) == (t // qch) else state["grp_n"]
                pbw = grp[1]
                n2 = min(2, nt - w0)
                nc.tensor.matmul(ot[:, 128:128 + n2 * 128], lT1,
                                 pbw[:, (w0 % qch) * 128:(w0 % qch + n2) * 128],
                                 start=True, stop=True,
                                 tile_position=(0, 0), skip_group_check=True)
                hpos2 = hppool.tile([128, 256], bf16, tag="hpos2",
                                    name=f"hpos2_{w0}")
                nc.scalar.activation(hpos2[:, 0:n2 * 128], ot[:, 128:128 + n2 * 128],
                                     Relu)
                return hpos2

            if t == 0:
                state["hpos2"] = stage1(0)
            if t % 2 == 0 and t > 0:
                state["hpos2"] = state["hpos2_n"]
            if t % 2 == 0 and t + 2 < nt:
                state["hpos2_n"] = stage1(t + 2)
            hpos = state["hpos2"][:, (t % 2) * 128:(t % 2) * 128 + 128]

            # ---- z8 = zk + zq + zp (own PSUM bank per buffer) ----
            z8 = pp_z.tile([128, 128], f32, tag="z8", name=f"z8_{t}")
            kbt = kb[:, (t % kch) * 512:(t % kch) * 512 + 512]
            for i in range(4):
                nc.tensor.matmul(
                    z8[32 * i:32 * i + 32, :], lZK,
                    kbt[:, 128 * i:128 * i + 128], start=True, stop=False,
                    tile_position=(0, 32 * i), skip_group_check=True)
            for i in range(4):
                qsl = qq[:, (t % qch) * 64 + 16 * i:(t % qch) * 64 + 16 * i + 16]
                qb = qsl.unsqueeze(2).broadcast_to([128, 16, 8])
                nc.tensor.matmul(
                    z8[32 * i:32 * i + 32, :], lZQ, qb, start=False, stop=False,
                    tile_position=(0, 32 * i), skip_group_check=True)
            nc.tensor.matmul(z8, lZP, hpos, start=False, stop=True,
                             tile_position=(0, 0), skip_group_check=True)

            h1 = h1pool.tile([128, 128], bf16, tag="h1")
            nc.scalar.activation(h1[:], z8, Relu)
            h1s[t] = h1

            # et tile for the pair; lg/exp/mult of tile t-1 are emitted here so
            # neither the PE queue (lg waiting on h1) nor Act (exp waiting on
            # lg) ever head-of-line-blocks younger independent work. Emitting
            # them before wv also hides wv's u-buffer WAR wait under the lg
            # matmuls.
            if p == 0:
                state["et"] = etpool.tile([128, 2048], fp16, tag="et",
                                          name=f"et{t//2}")
            ets[g] = state["et"]
            emit_lg_exp_mult(t - 1)

            # ---- u = k@Wv.T + hpos@pos_w2 (scaled by SU) ----
            u = pp_u.tile([128, 512], f32, tag="u", name=f"u{t}")
            us[t] = u
            kat = ka[:, (t % kch) * 1024:(t % kch) * 1024 + 1024]
            kav = kat.rearrange("p (s i j) -> p s i j", s=2, i=4)
            nc.tensor.matmul(u[:], lWV, kav,
                             start=True, stop=False, perf_mode=DR,
                             tile_position=(0, 0), skip_group_check=True)
            for i in range(4):
                nc.tensor.matmul(
                    u[:, 128 * i:128 * i + 128],
                    cons[:, C_UP + 128 * i:C_UP + 128 * i + 128],
                    hpos, start=False, stop=True,
                    tile_position=(0, 0), skip_group_check=True)

        def emit_lg_exp_mult(t):
            if t < 0:
                return
            h1 = h1s.pop(t)
            lg = pp_lg.tile([128, 512], f32, tag="lg")
            for i in range(4):
                nc.tensor.matmul(
                    lg[:, 128 * i:128 * i + 128],
                    cons[:, C_W3 + 128 * i:C_W3 + 128 * i + 128],
                    h1[:], start=True, stop=True,
                    tile_position=(0, 0), skip_group_check=True)
            et = ets[t // 2]
            p = t % 2
            nc.scalar.activation(et[:, p * 512:p * 512 + 512],
                                 lg[:], Exp, bias=b_exp)
            u = us.pop(t)
            with nc.allow_low_precision(reason="fp16 softmax tail"):
                nc.vector.tensor_tensor(et[:, 1024 + p * 512:1536 + p * 512],
                                        u[:], et[:, p * 512:p * 512 + 512],
                                        mult)

        def tail_a(g, et):
            """Softmax tail (elementwise, post-mult) for tile pair g -> x.
            The final pair runs dn/x on DVE (idle by then, and ~1us faster
            than Pool) to shorten the drain."""
            last = g == nt // 2 - 1
            eng = nc.vector if last else nc.gpsimd
            with nc.allow_low_precision(reason="fp16 softmax tail"):
                etv = et[:].rearrange("p (gp v) -> p gp v", v=8)
                t1 = t1pool.tile([128, 1024], fp16, tag="t1")
                t1v = t1[:].rearrange("p (gp v) -> p gp v", v=4)
                nc.vector.tensor_tensor(t1v, etv[:, :, 0:4], etv[:, :, 4:8], add)
                t2 = t2pool.tile([128, 512], fp16, tag="t2")
                t2v = t2[:].rearrange("p (gp v) -> p gp v", v=2)
                nc.vector.tensor_tensor(t2v, t1v[:, :, 0:2], t1v[:, :, 2:4], add)
                dn = dnpool.tile([128, 256], f32, tag="dn")
                dnv = dn[:].rearrange("p (gp v) -> p gp v", v=1)
                eng.tensor_tensor(dnv, t2v[:, :, 0:1], t2v[:, :, 1:2], add)
            rden = rxpool.tile([128, 128], f32, tag="rden")
            nc.vector.reciprocal_approx_fast(rden[:], dn[:, 0:128])
            x = rxpool.tile([128, 128], bf16, tag="x")
            eng.tensor_tensor(x[:], dn[:, 128:256], rden[:], mult)
            return x

        def tail_b(g, x, ob, obn):
            """Output matmul + bias + DMA for tile pair g."""
            nc.tensor.matmul(ot[:, 0:128], lOW, x[:], start=True, stop=True,
                             tile_position=(0, 0), skip_group_check=True)
            nc.scalar.activation(ob[:, (g % 4) * 128:(g % 4) * 128 + 128],
                                 ot[:, 0:128], Ident, bias=b_out)
            last = 2 * g + 1 == nt - 1
            lastgrp = (g // 4) == (nt - 1) // 8
            if lastgrp and g % 4 == 1 and obn >= 4:
                g0 = (g // 4) * 8
                nc.sync.dma_start(outT_d[:, g0 * 64:(g0 + 4) * 64],
                                  ob[:, 0:256])
            if lastgrp and g % 4 == 2 and obn >= 6:
                g0 = (g // 4) * 8
                nc.sync.dma_start(outT_d[:, (g0 + 4) * 64:(g0 + 6) * 64],
                                  ob[:, 256:384])
            if g % 4 == 3 or last:
                g0 = (g // 4) * 8
                if lastgrp and obn >= 6:
                    nc.sync.dma_start(
                        outT_d[:, (g0 + 6) * 64:(g0 + obn) * 64],
                        ob[:, 384:obn * 64])
                elif lastgrp and obn >= 4:
                    nc.sync.dma_start(
                        outT_d[:, (g0 + 4) * 64:(g0 + obn) * 64],
                        ob[:, 256:obn * 64])
                else:
                    nc.sync.dma_start(outT_d[:, g0 * 64:(g0 + obn) * 64],
                                      ob[:, 0:obn * 64])

        # ---- software-pipelined main loop ----
        # front(t) emits exp(t-1); tail_a(g) runs once exp(2g+1) is emitted
        # (at t=2g+2); tail_b (out matmul) trails 2 more pairs so the PE
        # queue never blocks on the elementwise tail.
        pend = []

        def run_tail_a(g):
            x = tail_a(g, ets.pop(g))
            ob, obn = obs[(2 * g) // qch]
            pend.append((g, x, ob, obn))
            if len(pend) >= 3:
                tail_b(*pend.pop(0))

        for t in range(nt):
            front(t)
            if t % 2 == 1 and t >= 3:
                run_tail_a((t - 3) // 2)
        emit_lg_exp_mult(nt - 1)
        run_tail_a(nt // 2 - 1)
        while pend:
            tail_b(*pend.pop(0))


# ----------------------------------------------------------------------------
# entry point
# ----------------------------------------------------------------------------

def kernel(q, k, pos, strength, q_tbl, k_tbl, v_tbl,
           pos_w1, pos_b1, pos_w2, pos_b2,
           attn_w1, attn_b1, attn_w2, attn_b2,
           out_w, out_b, str_w, str_b, mask, embed_id1) -> np.ndarray:
    from concourse.bass_utils import run_bass_kernel_spmd

    inputs = dict(q=q, k=k, pos=pos, strength=strength, q_tbl=q_tbl,
                  k_tbl=k_tbl, v_tbl=v_tbl, pos_w1=pos_w1, pos_b1=pos_b1,
                  pos_w2=pos_w2, pos_b2=pos_b2, attn_w1=attn_w1,
                  attn_b1=attn_b1, attn_w2=attn_w2, attn_b2=attn_b2,
                  out_w=out_w, out_b=out_b, str_w=str_w, str_b=str_b,
                  mask=mask, embed_id1=embed_id1)
    nc = build_program(NT_FULL)
    consts, consts8, biasf = make_consts(inputs)
    mask_f = np.asarray(mask).astype(np.float32)
    in_maps = []
    for c in range(NCORES):
        m = prep_core(inputs["q"], inputs["k"], inputs["pos"], mask_f, c)
        m["consts"] = consts
        m["consts8"] = consts8
        m["biasf"] = biasf
        in_maps.append(m)
    res = run_bass_kernel_spmd(nc, in_maps, core_ids=list(range(NCORES)))
    out = np.empty((B * N, DIM), np.float32)
    for c in range(NCORES):
        out[c * R_C:(c + 1) * R_C] = unprep_out(res.results[c]["outT"])
    return out.reshape(B, N, DIM)
